# revision 13
# baseline (speedup 1.0000x reference)
"""Trainium2 Bass kernel for nn_Attention_31396210933853.

Computation (B=32, S=4096, D=512):
    eij[b,s] = sum_d x[b,s,d]*kernel[d] + bias[s]
    a        = exp(tanh(eij)) * mask
    out[b,d] = sum_s a[b,s]*x[b,s,d] / (sum_s a[b,s] + EPS)

Single pass over x (normalization deferred): U = sum a*x and den =
sum a accumulate together, out = U/(den+EPS).  x is read from HBM
exactly once -> memory-bound at the ~333 GB/s effective per-core DMA
rate (32 MiB/core => ~100us floor).

Sharding: data-parallel over batch, 4 samples per core on 8 cores.

Layout: per sample, S=4096 splits into NG=4 groups of 1024 positions;
group tile (128, 8*512) holds s = g*1024 + p*8 + j at partition p,
free offset j*512+d.  Each group loads as 2 half-DMAs (8 KiB/partition
descriptors).  Per group:
  DVE/GpS: 8 scalar_tensor_tensor (x*k, fused free-dim reduce via
       accum_out) -> eraw (128,8); j-split between the two engines so
       neither exceeds the DMA rate.
  DVE : + bias (one (128,8) op)
  ACT : tanh, exp (batched (128,8))
  GpS : * mask -> a (128,8) in fp32r
  PE  : 8 matmuls a_j^T @ x_seg_j -> U psum (1,512)/sample (fp32r,
        1 cyc/row) + ones^T @ a -> den column slice (start=stop).
Finalize per sample: reduce den cols + EPS, reciprocal, U*rec, 2 KiB
DMA out.  Batching the pointwise chain at group granularity removes
the per-tile tiny-op overhead that made DVE/GpS/ACT the bottleneck
(837ns STT x128 + ~350ns tiny ops x384 in the per-tile version).
"""
import numpy as np

import concourse.bass as bass
import concourse.bacc as bacc
import concourse.tile as tile
from concourse import mybir
from concourse.bass_utils import run_bass_kernel_spmd

B, S, D = 32, 4096, 512
N_CORES = 8
BC = B // N_CORES        # samples per core
P = 128                  # SBUF partitions
GRP = 8                  # s-rows per partition per group
NG = S // (P * GRP)      # groups per sample (4)
HALF = GRP // 2 * D      # free-size of a half-group tile (2048)
EPS = 1e-7

# j-columns computed on GpSimd (rest on DVE). Half 0 owns j 0-3,
# half 1 owns j 4-7; keep the split roughly even per half. GpSimd has
# no scalar_tensor_tensor on TRN2, so its columns cost a tensor_tensor
# multiply + tensor_reduce (~2x a DVE STT) -> give it fewer columns.
GPS_J = (3, 6, 7)
XBUFS = 14               # half-group tile pipeline depth (8 KiB/partition each)

PASS_B_FP32R = True

# Set by a driver (e.g. test harness) to profile; off by default.
TRACE = False
LAST_RESULTS = None

_PROGRAM_CACHE = {}


def _build_program(fp32r: bool):
    f32 = mybir.dt.float32
    f32r = mybir.dt.float32r
    FT = mybir.ActivationFunctionType
    OP = mybir.AluOpType

    nc = bacc.Bacc(
        "TRN2", target_bir_lowering=False, debug=False, num_devices=N_CORES
    )
    xdt = f32r if fp32r else f32
    x_d = nc.dram_tensor("x", [BC, NG, P, GRP * D], xdt, kind="ExternalInput")
    kb_d = nc.dram_tensor("kb", [1, D], f32, kind="ExternalInput")
    bias_d = nc.dram_tensor("bias_t", [P, NG * GRP], f32, kind="ExternalInput")
    mask_d = nc.dram_tensor("mask_t", [BC, P, NG * GRP], f32, kind="ExternalInput")
    ones_d = nc.dram_tensor("ones", [P, 1], xdt, kind="ExternalInput")
    out_d = nc.dram_tensor("out", [1, BC * D], f32, kind="ExternalOutput")

    with tile.TileContext(nc) as tc:
        with (
            tc.tile_pool(name="xp", bufs=XBUFS) as xp,
            tc.tile_pool(name="cons", bufs=1) as cons,
            tc.tile_pool(name="tmpd", bufs=3) as tmpd,
            tc.tile_pool(name="tmpg", bufs=3) as tmpg,
            tc.tile_pool(name="tmpa", bufs=2) as tmpa,
            tc.tile_pool(name="small", bufs=8) as small,
            tc.tile_pool(name="fin", bufs=4) as fin,
            tc.tile_pool(name="psum", bufs=1, space="PSUM") as psp,
        ):
            # kb rides the sync HWDGE ring (ahead of the x tiles) so the
            # first STT isn't gated on the slower SWDGE path.
            kb = cons.tile([P, D], f32)
            nc.sync.dma_start(out=kb, in_=kb_d.ap().to_broadcast([P, D]))
            bias_t = cons.tile([P, NG * GRP], f32)
            nc.gpsimd.dma_start(out=bias_t, in_=bias_d[:])
            mask_all = cons.tile([P, BC * NG * GRP], f32)
            for b in range(BC):
                nc.gpsimd.dma_start(
                    out=mask_all[:, b * NG * GRP : (b + 1) * NG * GRP],
                    in_=mask_d[b],
                )
            ones = cons.tile([P, 1], xdt)
            nc.gpsimd.dma_start(out=ones, in_=ones_d[:])
            out_row = cons.tile([1, BC * D], f32)

            u_ps = [
                psp.tile([1, D], f32, name=f"u_ps{b}", tag=f"u{b}")
                for b in range(BC)
            ]
            den_ps = psp.tile([1, BC * NG * GRP], f32, tag="den")

            def _finalize(b):
                # Runs well after sample b's last matmul (emission is
                # deferred into the next sample) so the in-order DVE queue
                # never stalls on the PE counter.  The out DMA rides the
                # scalar ring: putting it on the sync ring would block
                # later x-tile dma_starts behind the whole sample pipeline.
                denr = fin.tile([1, 1], f32, tag="denr", name=f"denr{b}")
                nc.vector.tensor_reduce(
                    out=denr,
                    in_=den_ps[:, b * NG * GRP : (b + 1) * NG * GRP],
                    axis=mybir.AxisListType.X,
                    op=OP.add,
                )
                deno = fin.tile([1, 1], f32, tag="deno", name=f"deno{b}")
                nc.vector.tensor_scalar_add(deno, denr, EPS)
                rec = fin.tile([1, 1], f32, tag="rec", name=f"rec{b}")
                nc.vector.reciprocal(rec, deno)
                nc.vector.tensor_scalar_mul(
                    out_row[:, b * D : (b + 1) * D], u_ps[b], rec
                )
                nc.scalar.dma_start(
                    out=out_d[:, b * D : (b + 1) * D],
                    in_=out_row[:, b * D : (b + 1) * D],
                )

            pending_fin = None
            for b in range(BC):
                for g in range(NG):
                    if pending_fin is not None and g == 2:
                        _finalize(pending_fin)
                        pending_fin = None
                    # Two half-group loads: 8 KiB/partition descriptors.
                    xh = [
                        xp.tile([P, HALF], xdt, name=f"xh{h}", tag="xh")
                        for h in range(2)
                    ]
                    for h in range(2):
                        nc.sync.dma_start(
                            out=xh[h],
                            in_=x_d[b, g][:, h * HALF : (h + 1) * HALF],
                        )

                    eraw = small.tile([P, GRP], f32)
                    for j in range(GRP):
                        h, jj = divmod(j, GRP // 2)
                        src = xh[h] if fp32r is False else xh[h].bitcast(f32)
                        if j in GPS_J:
                            # Pool engine lacks scalar_tensor_tensor and
                            # free-axis tensor_reduce: multiply on GpSimd,
                            # reduce via ACT's accumulator (Copy+accum).
                            tmp = tmpg.tile([P, D], f32, name="tmpg", tag="tg")
                            nc.gpsimd.tensor_mul(
                                tmp, src[:, jj * D : (jj + 1) * D], kb
                            )
                            tmp2 = tmpa.tile([P, D], f32, name="tmpa", tag="ta")
                            nc.scalar.activation(
                                tmp2,
                                tmp,
                                FT.Copy,
                                accum_out=eraw[:, j : j + 1],
                            )
                        else:
                            tmp = tmpd.tile([P, D], f32, name="tmpd", tag="td")
                            nc.vector.scalar_tensor_tensor(
                                out=tmp,
                                in0=src[:, jj * D : (jj + 1) * D],
                                scalar=0.0,
                                in1=kb,
                                op0=OP.bypass,
                                op1=OP.mult,
                                accum_out=eraw[:, j : j + 1],
                            )

                    c0 = g * GRP
                    eij = small.tile([P, GRP], f32)
                    nc.vector.tensor_add(eij, eraw, bias_t[:, c0 : c0 + GRP])
                    th = small.tile([P, GRP], f32)
                    nc.scalar.activation(th, eij, FT.Tanh)
                    ex = small.tile([P, GRP], f32)
                    nc.scalar.activation(ex, th, FT.Exp)
                    a_m = small.tile([P, GRP], xdt)
                    nc.gpsimd.tensor_mul(
                        a_m, ex, mask_all[:, b * NG * GRP + c0 : b * NG * GRP + c0 + GRP]
                    )

                    for j in range(GRP):
                        h, jj = divmod(j, GRP // 2)
                        nc.tensor.matmul(
                            u_ps[b][:, :],
                            lhsT=a_m[:, j : j + 1],
                            rhs=xh[h][:, jj * D : (jj + 1) * D],
                            start=(g == 0 and j == 0),
                            stop=(g == NG - 1 and j == GRP - 1),
                        )
                    nc.tensor.matmul(
                        den_ps[:, b * NG * GRP + c0 : b * NG * GRP + c0 + GRP],
                        lhsT=ones,
                        rhs=a_m,
                        start=True,
                        stop=True,
                    )
                pending_fin = b
            _finalize(BC - 1)

    nc.compile()
    return nc


def _get_program(fp32r: bool):
    if fp32r not in _PROGRAM_CACHE:
        _PROGRAM_CACHE[fp32r] = _build_program(fp32r)
    return _PROGRAM_CACHE[fp32r]


def _prep_inputs(x, kern, bias, mask):
    """Host-side sharding/layout marshaling (views + tiny transposes only)."""
    x = np.ascontiguousarray(x, dtype=np.float32)
    kern = np.asarray(kern, dtype=np.float32)
    bias = np.asarray(bias, dtype=np.float32)
    kb = np.ascontiguousarray(kern[None, :])
    bias_t = np.ascontiguousarray(
        bias.reshape(NG, P, GRP).transpose(1, 0, 2).reshape(P, NG * GRP)
    )
    mask_f = np.asarray(mask).astype(np.float32)
    in_maps = []
    for i in range(N_CORES):
        xs = x[i * BC : (i + 1) * BC].reshape(BC, NG, P, GRP * D)
        ms = (
            mask_f[i * BC : (i + 1) * BC]
            .reshape(BC, NG, P, GRP)
            .transpose(0, 2, 1, 3)
            .reshape(BC, P, NG * GRP)
        )
        in_maps.append(
            {
                "x": xs,
                "kb": kb,
                "bias_t": bias_t,
                "mask_t": np.ascontiguousarray(ms),
                "ones": np.ones((P, 1), dtype=np.float32),
            }
        )
    return in_maps


def kernel(x, kernel, bias, mask):
    global LAST_RESULTS
    nc = _get_program(PASS_B_FP32R)
    in_maps = _prep_inputs(x, kernel, bias, mask)
    res = run_bass_kernel_spmd(nc, in_maps, list(range(N_CORES)), trace=TRACE)
    LAST_RESULTS = res
    out = np.concatenate(
        [res.results[i]["out"].reshape(BC, D) for i in range(N_CORES)], axis=0
    )
    return out.astype(np.float32, copy=False)


# revision 14
# speedup vs baseline: 1.3610x; 1.3610x over previous
"""Trainium2 Bass kernel for nn_Attention_31396210933853.

Computation (B=32, S=4096, D=512):
    eij[b,s] = sum_d x[b,s,d]*kernel[d] + bias[s]
    a        = exp(tanh(eij)) * mask
    out[b,d] = sum_s a[b,s]*x[b,s,d] / (sum_s a[b,s] + EPS)

Single pass over x (normalization deferred): U = sum a*x and den =
sum a accumulate together, out = U/(den+EPS).  x is read from HBM
exactly once -> memory-bound at the ~333 GB/s effective per-core DMA
rate (32 MiB/core => ~100us floor).

Sharding: data-parallel over batch, 4 samples per core on 8 cores.

Layout: per sample, S=4096 splits into NG=4 groups of 1024 positions;
group tile (128, 8*512) holds s = g*1024 + p*8 + j at partition p,
free offset j*512+d.  Each group loads as 2 half-DMAs (8 KiB/partition
descriptors).  Per group:
  DVE/GpS: 8 scalar_tensor_tensor (x*k, fused free-dim reduce via
       accum_out) -> eraw (128,8); j-split between the two engines so
       neither exceeds the DMA rate.
  DVE : + bias (one (128,8) op)
  ACT : tanh, exp (batched (128,8))
  GpS : * mask -> a (128,8) in fp32r
  PE  : 8 matmuls a_j^T @ x_seg_j -> U psum (1,512)/sample (fp32r,
        1 cyc/row) + ones^T @ a -> den column slice (start=stop).
Finalize per sample: reduce den cols + EPS, reciprocal, U*rec, 2 KiB
DMA out.  Batching the pointwise chain at group granularity removes
the per-tile tiny-op overhead that made DVE/GpS/ACT the bottleneck
(837ns STT x128 + ~350ns tiny ops x384 in the per-tile version).
"""
import numpy as np

import concourse.bass as bass
import concourse.bacc as bacc
import concourse.tile as tile
from concourse import mybir
from concourse.bass_utils import run_bass_kernel_spmd

B, S, D = 32, 4096, 512
N_CORES = 8
BC = B // N_CORES        # samples per core
P = 128                  # SBUF partitions
GRP = 8                  # s-rows per partition per group
NG = S // (P * GRP)      # groups per sample (4)
HALF = GRP // 2 * D      # free-size of a half-group tile (2048)
EPS = 1e-7

# j-columns computed on GpSimd (rest on DVE). Half 0 owns j 0-3,
# half 1 owns j 4-7; keep the split roughly even per half. GpSimd has
# no scalar_tensor_tensor on TRN2, so its columns cost a tensor_tensor
# multiply + tensor_reduce (~2x a DVE STT) -> give it fewer columns.
GPS_J = (3, 7)
XBUFS = 14               # half-group tile pipeline depth (8 KiB/partition each)

PASS_B_FP32R = True

# Set by a driver (e.g. test harness) to profile; off by default.
TRACE = False
LAST_RESULTS = None

_PROGRAM_CACHE = {}


def _build_program(fp32r: bool):
    f32 = mybir.dt.float32
    f32r = mybir.dt.float32r
    FT = mybir.ActivationFunctionType
    OP = mybir.AluOpType

    nc = bacc.Bacc(
        "TRN2", target_bir_lowering=False, debug=False, num_devices=N_CORES
    )
    xdt = f32r if fp32r else f32
    x_d = nc.dram_tensor("x", [BC, NG, P, GRP * D], xdt, kind="ExternalInput")
    kb_d = nc.dram_tensor("kb", [1, D], f32, kind="ExternalInput")
    bias_d = nc.dram_tensor("bias_t", [P, NG * GRP], f32, kind="ExternalInput")
    mask_d = nc.dram_tensor("mask_t", [BC, P, NG * GRP], f32, kind="ExternalInput")
    ones_d = nc.dram_tensor("ones", [P, 1], xdt, kind="ExternalInput")
    out_d = nc.dram_tensor("out", [1, BC * D], f32, kind="ExternalOutput")

    with tile.TileContext(nc) as tc:
        with (
            tc.tile_pool(name="xp", bufs=XBUFS) as xp,
            tc.tile_pool(name="cons", bufs=1) as cons,
            tc.tile_pool(name="tmpd", bufs=3) as tmpd,
            tc.tile_pool(name="tmpg", bufs=3) as tmpg,
            tc.tile_pool(name="tmpa", bufs=2) as tmpa,
            tc.tile_pool(name="small", bufs=8) as small,
            tc.tile_pool(name="fin", bufs=4) as fin,
            tc.tile_pool(name="psum", bufs=1, space="PSUM") as psp,
        ):
            # kb rides the sync HWDGE ring (ahead of the x tiles) so the
            # first STT isn't gated on the slower SWDGE path.
            kb = cons.tile([P, D], f32)
            nc.sync.dma_start(out=kb, in_=kb_d.ap().to_broadcast([P, D]))
            bias_t = cons.tile([P, NG * GRP], f32)
            nc.gpsimd.dma_start(out=bias_t, in_=bias_d[:])
            mask_all = cons.tile([P, BC * NG * GRP], f32)
            for b in range(BC):
                nc.gpsimd.dma_start(
                    out=mask_all[:, b * NG * GRP : (b + 1) * NG * GRP],
                    in_=mask_d[b],
                )
            ones = cons.tile([P, 1], xdt)
            nc.gpsimd.dma_start(out=ones, in_=ones_d[:])
            out_row = cons.tile([1, BC * D], f32)

            u_ps = [
                psp.tile([1, D], f32, name=f"u_ps{b}", tag=f"u{b}")
                for b in range(BC)
            ]
            den_ps = psp.tile([1, BC * NG * GRP], f32, tag="den")

            def _finalize(b):
                # Runs well after sample b's last matmul (emission is
                # deferred into the next sample) so the in-order DVE queue
                # never stalls on the PE counter.  The out DMA rides the
                # scalar ring: putting it on the sync ring would block
                # later x-tile dma_starts behind the whole sample pipeline.
                denr = fin.tile([1, 1], f32, tag="denr", name=f"denr{b}")
                nc.vector.tensor_reduce(
                    out=denr,
                    in_=den_ps[:, b * NG * GRP : (b + 1) * NG * GRP],
                    axis=mybir.AxisListType.X,
                    op=OP.add,
                )
                deno = fin.tile([1, 1], f32, tag="deno", name=f"deno{b}")
                nc.vector.tensor_scalar_add(deno, denr, EPS)
                rec = fin.tile([1, 1], f32, tag="rec", name=f"rec{b}")
                nc.vector.reciprocal(rec, deno)
                nc.vector.tensor_scalar_mul(
                    out_row[:, b * D : (b + 1) * D], u_ps[b], rec
                )
                nc.scalar.dma_start(
                    out=out_d[:, b * D : (b + 1) * D],
                    in_=out_row[:, b * D : (b + 1) * D],
                )

            pending_fin = None
            for b in range(BC):
                for g in range(NG):
                    if pending_fin is not None and g == 2:
                        _finalize(pending_fin)
                        pending_fin = None
                    # Two half-group loads: 8 KiB/partition descriptors.
                    xh = [
                        xp.tile([P, HALF], xdt, name=f"xh{h}", tag="xh")
                        for h in range(2)
                    ]
                    for h in range(2):
                        nc.sync.dma_start(
                            out=xh[h],
                            in_=x_d[b, g][:, h * HALF : (h + 1) * HALF],
                        )

                    eraw = small.tile([P, GRP], f32)
                    for j in range(GRP):
                        h, jj = divmod(j, GRP // 2)
                        src = xh[h] if fp32r is False else xh[h].bitcast(f32)
                        if j in GPS_J:
                            # Pool engine lacks scalar_tensor_tensor and
                            # free-axis tensor_reduce: multiply on GpSimd,
                            # reduce via ACT's accumulator (Copy+accum).
                            tmp = tmpg.tile([P, D], f32, name="tmpg", tag="tg")
                            nc.gpsimd.tensor_mul(
                                tmp, src[:, jj * D : (jj + 1) * D], kb
                            )
                            tmp2 = tmpa.tile([P, D], f32, name="tmpa", tag="ta")
                            nc.scalar.activation(
                                tmp2,
                                tmp,
                                FT.Copy,
                                accum_out=eraw[:, j : j + 1],
                            )
                        else:
                            tmp = tmpd.tile([P, D], f32, name="tmpd", tag="td")
                            nc.vector.scalar_tensor_tensor(
                                out=tmp,
                                in0=src[:, jj * D : (jj + 1) * D],
                                scalar=0.0,
                                in1=kb,
                                op0=OP.bypass,
                                op1=OP.mult,
                                accum_out=eraw[:, j : j + 1],
                            )

                    c0 = g * GRP
                    eij = small.tile([P, GRP], f32)
                    nc.vector.tensor_add(eij, eraw, bias_t[:, c0 : c0 + GRP])
                    th = small.tile([P, GRP], f32)
                    nc.scalar.activation(th, eij, FT.Tanh)
                    ex = small.tile([P, GRP], f32)
                    nc.scalar.activation(ex, th, FT.Exp)
                    a_m = small.tile([P, GRP], xdt)
                    nc.gpsimd.tensor_mul(
                        a_m, ex, mask_all[:, b * NG * GRP + c0 : b * NG * GRP + c0 + GRP]
                    )

                    for j in range(GRP):
                        h, jj = divmod(j, GRP // 2)
                        nc.tensor.matmul(
                            u_ps[b][:, :],
                            lhsT=a_m[:, j : j + 1],
                            rhs=xh[h][:, jj * D : (jj + 1) * D],
                            start=(g == 0 and j == 0),
                            stop=(g == NG - 1 and j == GRP - 1),
                        )
                    nc.tensor.matmul(
                        den_ps[:, b * NG * GRP + c0 : b * NG * GRP + c0 + GRP],
                        lhsT=ones,
                        rhs=a_m,
                        start=True,
                        stop=True,
                    )
                pending_fin = b
            _finalize(BC - 1)

    nc.compile()
    return nc


def _get_program(fp32r: bool):
    if fp32r not in _PROGRAM_CACHE:
        _PROGRAM_CACHE[fp32r] = _build_program(fp32r)
    return _PROGRAM_CACHE[fp32r]


def _prep_inputs(x, kern, bias, mask):
    """Host-side sharding/layout marshaling (views + tiny transposes only)."""
    x = np.ascontiguousarray(x, dtype=np.float32)
    kern = np.asarray(kern, dtype=np.float32)
    bias = np.asarray(bias, dtype=np.float32)
    kb = np.ascontiguousarray(kern[None, :])
    bias_t = np.ascontiguousarray(
        bias.reshape(NG, P, GRP).transpose(1, 0, 2).reshape(P, NG * GRP)
    )
    mask_f = np.asarray(mask).astype(np.float32)
    in_maps = []
    for i in range(N_CORES):
        xs = x[i * BC : (i + 1) * BC].reshape(BC, NG, P, GRP * D)
        ms = (
            mask_f[i * BC : (i + 1) * BC]
            .reshape(BC, NG, P, GRP)
            .transpose(0, 2, 1, 3)
            .reshape(BC, P, NG * GRP)
        )
        in_maps.append(
            {
                "x": xs,
                "kb": kb,
                "bias_t": bias_t,
                "mask_t": np.ascontiguousarray(ms),
                "ones": np.ones((P, 1), dtype=np.float32),
            }
        )
    return in_maps


def kernel(x, kernel, bias, mask):
    global LAST_RESULTS
    nc = _get_program(PASS_B_FP32R)
    in_maps = _prep_inputs(x, kernel, bias, mask)
    res = run_bass_kernel_spmd(nc, in_maps, list(range(N_CORES)), trace=TRACE)
    LAST_RESULTS = res
    out = np.concatenate(
        [res.results[i]["out"].reshape(BC, D) for i in range(N_CORES)], axis=0
    )
    return out.astype(np.float32, copy=False)
